# revision 29
# baseline (speedup 1.0000x reference)
"""Bass/Trainium2 kernel for nn_EpisodicMemory (8-core data-parallel).

Strategy
--------
Batch (16384 rows) is sharded across 8 NeuronCores (2048 rows each, 16
row-tiles of 128).  All O(weights) transforms happen on the host; all O(B)
work happens on-device:

 - qk^T      = key_w @ x^T                      (PE, bf16, per row-tile)
 - sim       = qk @ kwp^T, scaled by 0.45/sqrt(KD)/||qk|| (PE + ACT/DVE)
 - attn      = softmax(clip(sim*s + bias, 0, 1))          (ACT/DVE, no max
               subtraction needed: sal is in [0,1])
 - retrieved = attn @ mem_vals                  (PE)
 - gate/h preacts fused into ONE accumulation:
       [gate|h] = x @ [Wgx^T|Wox^T] + attn @ [Mg|Mo] + [gate_b|out_b]
   where Mg = mem_vals @ Wgr^T etc. are tiny host-precomputed [64,1024]
   matrices (this removes the retrieved-half of the 2048-wide contraction:
   ~1.75x fewer PE flops), and the bias rides in as a ones-row of attn^T.
 - epilogue: sigmoid/gelu (ACT), blended + LayerNorm (DVE/ACT).

x is shipped twice: natural fp32 (elementwise path) and pre-transposed
bf16 (matmul lhsT path) - a host-side layout transform.
"""

import math
import sys

import numpy as np

try:
    import concourse.bass as bass
except ImportError:  # harness runs from a fresh dir; repo is baked in the image
    sys.path.insert(0, "/opt/trn_rl_repo")
    import concourse.bass as bass

import ml_dtypes

import concourse.mybir as mybir
import concourse.tile as tile
from concourse.bass_utils import run_bass_kernel_spmd
from concourse.masks import make_identity

# ---------------------------------------------------------------- constants
HID = 1024
SLOTS = 64
KD = 32
B = 16384
NCORES = 8
R = B // NCORES          # rows per core
P = 128                  # partitions
NT = R // P              # row-tiles per core
KO = HID // P            # k-chunks of the HID contraction
NB = 512                 # psum chunk width (one bank of fp32)
SCALE = 0.45 / math.sqrt(KD)

F32 = mybir.dt.float32
BF16 = mybir.dt.bfloat16
npbf16 = ml_dtypes.bfloat16

AF = mybir.ActivationFunctionType
OP = mybir.AluOpType

_nc_cache = {}


# ---------------------------------------------------------------- device IR
MAX_WAITS = 1


def _split_excess_waits(nc: bass.Bass, max_waits: int = MAX_WAITS):
    """This container's walrus build accepts only a couple of sem-wait slots
    per instruction ("Too many sync wait commands"), while Tile's
    sem-assigner happily attaches one wait per producer proc.  Hoist excess
    waits onto preceding NOPs on the same engine (engines execute their
    stream in order, so semantics are unchanged)."""
    n_split = 0
    for fn in nc.m.functions:
        for blk in fn.blocks:
            insts = list(blk.instructions)
            new = []
            changed = False
            for ins in insts:
                si = getattr(ins, "sync_info", None)
                waits = list(si.on_wait) if si is not None and si.on_wait else []
                if len(waits) > max_waits:
                    extra, keep = waits[:-max_waits], waits[-max_waits:]
                    for j in range(0, len(extra), max_waits):
                        nop = mybir.InstNoOp(
                            name=f"{ins.name}-w{j}",
                            engine=ins.engine,
                            bass_nofuse=True,
                            sync_info=mybir.SyncInfo(
                                on_wait=extra[j:j + max_waits], on_update=[]
                            ),
                        )
                        new.append(nop)
                    si.on_wait = keep
                    changed = True
                    n_split += 1
                new.append(ins)
            if changed:
                blk.instructions = new
    return n_split


def _build(has_affine: bool, repeat: int = 1) -> bass.Bass:
    nc = bass.Bass()

    x32_d = nc.dram_tensor("x32", [R, HID], F32, kind="ExternalInput")
    xbt_d = nc.dram_tensor("xbt", [HID, R], BF16, kind="ExternalInput")
    wbig_d = nc.dram_tensor("wbig", [HID, 2 * HID], BF16, kind="ExternalInput")
    kwt_d = nc.dram_tensor("kwt", [HID, KD], BF16, kind="ExternalInput")
    vm_d = nc.dram_tensor("vm", [SLOTS + 1, 3 * HID], BF16, kind="ExternalInput")
    kwp_d = nc.dram_tensor("kwp", [KD, SLOTS], BF16, kind="ExternalInput")
    bias_d = nc.dram_tensor("biasv", [SLOTS], F32, kind="ExternalInput")
    if has_affine:
        ln_d = nc.dram_tensor("lnw", [2, HID], F32, kind="ExternalInput")
    y_d = nc.dram_tensor("y", [R, HID], F32, kind="ExternalOutput")

    with tile.TileContext(nc) as tc:
        with (
            tc.tile_pool(name="consts", bufs=1) as consts,
            tc.tile_pool(name="inx", bufs=3) as inx,
            tc.tile_pool(name="work", bufs=2) as work,
            tc.tile_pool(name="small", bufs=3) as small,
            tc.tile_pool(name="outp", bufs=3) as outp,
            tc.tile_pool(name="ps_small", bufs=1, space="PSUM") as ps_small,
            tc.tile_pool(name="ps_retr", bufs=1, space="PSUM") as ps_retr,
            tc.tile_pool(name="ps_chunk", bufs=5, space="PSUM") as ps_chunk,
        ):
            # ---- resident constants -------------------------------------
            kw_s = consts.tile([P, KO, KD], BF16)
            nc.sync.dma_start(kw_s, kwt_d[:, :].rearrange("(ko p) n -> p ko n", p=P))

            xbt_s = consts.tile([P, KO, R], BF16)
            xbt_ap = xbt_d[:, :].rearrange("(ko p) r -> p ko r", p=P)
            wbig_s = consts.tile([P, KO, 2 * HID], BF16)
            wbig_ap = wbig_d[:, :].rearrange("(ko p) n -> p ko n", p=P)
            for ko in range(KO):
                nc.sync.dma_start(xbt_s[:, ko, :], xbt_ap[:, ko, :])
            for ko in range(KO):
                nc.sync.dma_start(wbig_s[:, ko, :], wbig_ap[:, ko, :])

            vm_s = consts.tile([SLOTS + 1, 3 * HID], BF16)
            nc.sync.dma_start(vm_s, vm_d[:, :])

            kwp_s = consts.tile([KD, SLOTS], BF16)
            nc.sync.dma_start(kwp_s, kwp_d[:, :])

            bias_s = consts.tile([P, SLOTS], F32)
            nc.gpsimd.dma_start(bias_s, bass.AP(bias_d, 0, [[0, P], [1, SLOTS]]))

            if has_affine:
                g_s = consts.tile([P, HID], F32)
                nc.gpsimd.dma_start(g_s, bass.AP(ln_d, 0, [[0, P], [1, HID]]))
                b_s = consts.tile([P, HID], F32)
                nc.gpsimd.dma_start(b_s, bass.AP(ln_d, HID, [[0, P], [1, HID]]))

            ident_s = consts.tile([P, P], BF16)
            make_identity(nc, ident_s)



            ones_s = consts.tile([KD, 1], BF16)
            nc.vector.memset(ones_s, 1.0)

            epsn_s = consts.tile([P, 1], F32)
            nc.vector.memset(epsn_s, 1e-24)
            epsl_s = consts.tile([P, 1], F32)
            nc.vector.memset(epsl_s, 1e-5)

            # ---- qk^T, batched in 512-row chunks, emitted lazily --------
            # qkT_full[kd, r] = sum_k key_w[kd, k] * x[r, k]
            qkt_full = consts.tile([KD, R], BF16, tag="qkt_full")
            sq_full = consts.tile([KD, R], BF16, tag="sq_full")
            qkt_done = set()

            def ensure_qkt_chunk(c4):
                if c4 in qkt_done:
                    return
                qkt_done.add(c4)
                sl = slice(c4 * NB, (c4 + 1) * NB)
                qkt_ps = ps_chunk.tile([KD, NB], F32, tag="psc")
                for k in range(KO):
                    nc.tensor.matmul(
                        qkt_ps, kw_s[:, k, :], xbt_s[:, k, sl],
                        start=(k == 0), stop=(k == KO - 1),
                    )
                nc.scalar.copy(qkt_full[:, sl], qkt_ps)
                nc.vector.tensor_mul(sq_full[:, sl], qkt_full[:, sl],
                                     qkt_full[:, sl])

            # ---- per row-tile pipeline (softmax runs one tile ahead) ----
            def softmax_a1(i):
                """ss/sim matmuls + exp chain -> normalized attn (bf16)."""
                ensure_qkt_chunk(i * P // NB)
                rows = slice(i * P, (i + 1) * P)
                ss_ps = ps_small.tile([P, 1], F32, tag="pss")
                nc.tensor.matmul(ss_ps, sq_full[:, rows], ones_s,
                                 start=True, stop=True)
                # s_r = SCALE / ||qk||
                srt = small.tile([P, 1], F32, tag="sm_srt")
                nc.scalar.activation(
                    srt, ss_ps, AF.Sqrt, bias=epsn_s, scale=1.0 / (SCALE * SCALE)
                )
                s_r = small.tile([P, 1], F32, tag="sm_sr")
                nc.vector.reciprocal(s_r, srt)

                sim_ps = ps_small.tile([P, SLOTS], F32, tag="pss")
                nc.tensor.matmul(sim_ps, qkt_full[:, rows], kwp_s,
                                 start=True, stop=True)

                t = small.tile([P, SLOTS], F32, tag="sm_t")
                nc.vector.tensor_scalar_mul(t, sim_ps, s_r)
                nc.vector.tensor_add(t, t, bias_s)
                nc.vector.tensor_scalar(t, t, 1.0, 0.0, OP.min, OP.max)
                p_t = small.tile([P, SLOTS], F32, tag="sm_p")
                denom = small.tile([P, 1], F32, tag="sm_den")
                nc.scalar.activation(p_t, t, AF.Exp, accum_out=denom)
                rec = small.tile([P, 1], F32, tag="sm_rec")
                nc.vector.reciprocal(rec, denom)
                attn_bf = small.tile([P, SLOTS], BF16, tag="sm_attn")
                nc.vector.tensor_scalar_mul(attn_bf, p_t, rec)
                return attn_bf

            def softmax_a2(attn_bf):
                """transpose attn -> [S+1, P] lhsT with a trailing ones row."""
                attnt_s = small.tile([SLOTS + 1, P], BF16, tag="sm_attnt")
                nc.vector.memset(attnt_s[SLOTS:SLOTS + 1, :], 1.0)
                attnt_ps = ps_small.tile([SLOTS, P], BF16, tag="pss")
                nc.tensor.transpose(attnt_ps, attn_bf, ident_s)
                nc.vector.tensor_copy(attnt_s[0:SLOTS, :], attnt_ps)
                return attnt_s

            # Software pipeline: softmax exp-chain (a1) runs two tiles ahead
            # (emitted after the epilogue so the chunk-releasing ACT reads
            # keep priority), the transpose (a2) one tile ahead.
            tiles = [t for _ in range(repeat) for t in range(NT)]
            attnt_q = [softmax_a2(softmax_a1(tiles[0]))]
            if len(tiles) > 1:
                attnt_q.append(softmax_a2(softmax_a1(tiles[1])))
            abf_pend = softmax_a1(tiles[2]) if len(tiles) > 2 else None
            for idx, i in enumerate(tiles):
                rows = slice(i * P, (i + 1) * P)

                x32 = inx.tile([P, HID], F32)
                nc.sync.dma_start(x32, x32_d[rows, :])

                xt = xbt_s[:, :, rows]  # [P, KO, P] view: lhsT chunks

                # gate/h x-part accumulation (attn-part lands later)
                gch = []
                for c in range(4):
                    pt = ps_chunk.tile([P, NB], F32, tag="psc")
                    for k in range(KO):
                        nc.tensor.matmul(
                            pt, xt[:, k, :], wbig_s[:, k, c * NB:(c + 1) * NB],
                            start=(k == 0), stop=False,
                        )
                    gch.append(pt)

                # ---- retrieval + attn-part of gate/h --------------------
                attnt_cur = attnt_q.pop(0)
                retr = ps_retr.tile([P, HID], F32, tag="psr")
                for c in range(2):
                    nc.tensor.matmul(
                        retr[:, c * NB:(c + 1) * NB], attnt_cur,
                        vm_s[:, c * NB:(c + 1) * NB],
                        start=True, stop=True,
                    )
                for c in range(4):
                    nc.tensor.matmul(
                        gch[c], attnt_cur, vm_s[:, HID + c * NB:HID + (c + 1) * NB],
                        start=False, stop=True,
                    )

                # ---- epilogue -------------------------------------------
                gate = work.tile([P, HID], F32, tag="gate")
                nc.scalar.activation(gate[:, 0:NB], gch[0], AF.Sigmoid)
                nc.scalar.activation(gate[:, NB:HID], gch[1], AF.Sigmoid)
                h = work.tile([P, HID], F32, tag="h")
                nc.scalar.activation(h[:, 0:NB], gch[2], AF.Gelu)
                nc.scalar.activation(h[:, NB:HID], gch[3], AF.Gelu)

                d = work.tile([P, HID], F32, tag="d")
                nc.vector.tensor_tensor(d, retr, x32, OP.subtract)
                gd = work.tile([P, HID], F32, tag="gd")
                nc.vector.tensor_mul(gd, gate, d)
                y1 = work.tile([P, HID], F32, tag="y1")
                nc.gpsimd.tensor_add(y1, h, x32)
                y = work.tile([P, HID], F32, tag="y")
                nc.vector.tensor_add(y, y1, gd)
                sum_y = small.tile([P, 1], F32, tag="sm_sumy")
                nc.vector.reduce_sum(sum_y, y, axis=mybir.AxisListType.X)

                # LayerNorm stats via E[y^2] - mu^2 (Square doesn't wait on mu)
                sqsum = small.tile([P, 1], F32, tag="sm_sqsum")
                nc.scalar.activation(gd, y, AF.Square, accum_out=sqsum)
                negmu = small.tile([P, 1], F32, tag="sm_negmu")
                nc.vector.tensor_scalar_mul(negmu, sum_y, -1.0 / HID)
                mu2 = small.tile([P, 1], F32, tag="sm_mu2")
                nc.vector.tensor_mul(mu2, negmu, negmu)
                eb = small.tile([P, 1], F32, tag="sm_eb")
                nc.vector.tensor_tensor(eb, epsl_s, mu2, OP.subtract)
                stdv = small.tile([P, 1], F32, tag="sm_std")
                nc.scalar.activation(
                    stdv, sqsum, AF.Sqrt, bias=eb, scale=1.0 / HID
                )
                rstd = small.tile([P, 1], F32, tag="sm_rstd")
                nc.vector.reciprocal(rstd, stdv)

                out_t = outp.tile([P, HID], F32)
                nc.vector.tensor_scalar(
                    out_t, y, negmu, rstd, OP.add, OP.mult
                )
                if has_affine:
                    nc.vector.tensor_mul(out_t, out_t, g_s)
                    nc.vector.tensor_add(out_t, out_t, b_s)

                nc.sync.dma_start(y_d[rows, :], out_t)

                # pipeline refill: transpose for tile idx+2, exp-chain for idx+3
                if abf_pend is not None:
                    attnt_q.append(softmax_a2(abf_pend))
                abf_pend = (
                    softmax_a1(tiles[idx + 3]) if idx + 3 < len(tiles) else None
                )

    _split_excess_waits(nc)
    return nc


def _get_nc(has_affine: bool) -> bass.Bass:
    key = has_affine
    if key not in _nc_cache:
        _nc_cache[key] = _build(has_affine)
    return _nc_cache[key]


# ---------------------------------------------------------------- host side
def _prep(x, key_w, out_w, out_b, gate_w, gate_b, ln_g, ln_b,
          pos_table, mem_keys, mem_vals, mem_age, mem_conf, slot_order):
    f32 = np.float32
    x = np.asarray(x, f32)
    key_w = np.asarray(key_w, f32)
    out_w = np.asarray(out_w, f32)
    out_b = np.asarray(out_b, f32)
    gate_w = np.asarray(gate_w, f32)
    gate_b = np.asarray(gate_b, f32)
    ln_g = np.asarray(ln_g, f32)
    ln_b = np.asarray(ln_b, f32)
    pos_table = np.asarray(pos_table, f32)
    mem_keys = np.asarray(mem_keys, f32)
    mem_vals = np.asarray(mem_vals, f32)
    mem_age = np.asarray(mem_age, f32)
    mem_conf = np.asarray(mem_conf, f32)
    slot_order = np.asarray(slot_order)

    has_affine = not (np.all(ln_g == 1.0) and np.all(ln_b == 0.0))

    wbig = np.concatenate(
        [gate_w[:, :HID].T, out_w[:, :HID].T], axis=1
    ).astype(npbf16)                                     # [HID, 2*HID]
    kwt = np.ascontiguousarray(key_w.T).astype(npbf16)   # [HID, KD]

    mg = mem_vals @ gate_w[:, HID:].T                    # [S, HID]
    mo = mem_vals @ out_w[:, HID:].T
    vm = np.zeros((SLOTS + 1, 3 * HID), f32)
    vm[:SLOTS, :HID] = mem_vals
    vm[:SLOTS, HID:2 * HID] = mg
    vm[:SLOTS, 2 * HID:] = mo
    vm[SLOTS, HID:2 * HID] = gate_b
    vm[SLOTS, 2 * HID:] = out_b
    vm = vm.astype(npbf16)

    pos_emb = pos_table[slot_order % SLOTS]
    kwp = mem_keys + f32(0.1) * pos_emb
    kwp = kwp / np.clip(
        np.linalg.norm(kwp, axis=-1, keepdims=True), 1e-12, None
    ).astype(f32)
    kwpt = np.ascontiguousarray(kwp.T).astype(npbf16)    # [KD, SLOTS]

    recency = np.exp(-mem_age / f32(200.0))
    freq = np.clip(mem_age, 1.0, None).astype(f32)
    freq_norm = np.log(freq + f32(1.0)) / (np.log(freq.max() + f32(2.0)) + f32(1e-8))
    biasv = (
        f32(0.2) * recency + f32(0.15) * freq_norm
        + f32(0.1) * mem_conf + f32(0.1) * f32(0.8)
    ).astype(f32)

    xbt = np.ascontiguousarray(x.T).astype(npbf16)       # [HID, B]

    lnw = np.stack([ln_g, ln_b]).astype(f32) if has_affine else None

    in_maps = []
    for c in range(NCORES):
        rs, re = c * R, (c + 1) * R
        m = {
            "x32": np.ascontiguousarray(x[rs:re]),
            "xbt": np.ascontiguousarray(xbt[:, rs:re]),
            "wbig": wbig,
            "kwt": kwt,
            "vm": vm,
            "kwp": kwpt,
            "biasv": biasv,
        }
        if has_affine:
            m["lnw"] = lnw
        in_maps.append(m)
    return in_maps, has_affine


def _run(trace=False, **inputs):
    in_maps, has_affine = _prep(**inputs)
    nc = _get_nc(has_affine)
    try:
        res = run_bass_kernel_spmd(
            nc, in_maps, core_ids=list(range(NCORES)), trace=trace
        )
    except Exception:
        # transient axon/NRT hiccups have been observed; one retry
        res = run_bass_kernel_spmd(
            nc, in_maps, core_ids=list(range(NCORES)), trace=trace
        )
    out = np.concatenate([res.results[c]["y"] for c in range(NCORES)], axis=0)
    return out, res


def kernel(**inputs) -> np.ndarray:
    out, _ = _run(trace=False, **inputs)
    return out
